# revision 4
# baseline (speedup 1.0000x reference)
"""Trainium2 kernel for PhysicalDiffraction:
    out = real(ifft2(fft2(x) * (H_real + i*H_imag)))   x: [8,16,512,512] f32

Method (Hartley / DHT formulation)
----------------------------------
The op is a real->real linear map: out = x (*) h_r with h_r =
real(ifft2(Hc)) a real circular kernel, i.e. the filter can be
Hermitian-symmetrized G = (Hc + conj(Hc[-u,-v]))/2 = fft2(h_r).

Using the separable row-column discrete Hartley transform
A = CASP @ x @ CASP (CASP = C + S, cas matrix), the convolution theorem
is   Z = A.E1 + A(-u,v).E2 + A(u,-v).E3 + A(-u,-v).E4
with E1..E4 host-precomputed from G.  Writing each reversed term as a
reversal of a plain product (Ru(A.Ru E2) etc.) and noting that a
u-reversal of a matmul's lhsT is equivalent to swapping the DFT weight
matrix CASP <-> CASM (CASM = C - S, since cas(-x) = cos - sin), ALL
index reversals get absorbed into which weight matrix the inverse
stages use.  Per image (all real matmuls, 512^3):

  S1: S1p = x^T CASP                          (1 unit)
  S2: A   = S1p^T CASP                        (1 unit)
  S3: m_i = A . E_i  (i=1..4, VectorE, fused PSUM evacuation)
  S4: Ta = m1^T CASP + m2^T CASM              (2 units)
      Tb = m3^T CASP + m4^T CASM
  S5: out = Ta^T CASP + Tb^T CASM             (2 units)

6 matmul units/image vs 8 for complex-packing -> ~25% less TensorE
work, no transposes, no cross-partition moves.  Matmuls in float32r
(full speed, ~1e-4 relative).  The lhsT=data trick makes each stage
transform + transpose in one go.

Sharding: batch*channel = 128 images, 16 per NeuronCore, data-parallel.

The walrus build here accepts only ONE semaphore wait per instruction;
Tile emits more.  `legalize_waits` splits excess waits onto same-engine
NoOps inserted just before the instruction (the engine sequencer stalls
there instead), which preserves semantics.
"""

import sys

for _p in ("/opt/trn_rl_repo", "/root/.axon_site/_ro/trn_rl_repo"):
    if _p not in sys.path:
        sys.path.append(_p)

import numpy as np
from concourse import bass, mybir
from concourse.tile import TileContext
from concourse.bass_utils import run_bass_kernel_spmd

N = 512
NCHUNK = N // 128  # 4
N_CORES = 8
IMG_PER_CORE = 16

MM_DTYPE = mybir.dt.float32r


# ---------------------------------------------------------------------------
# wait legalizer
# ---------------------------------------------------------------------------
_counter = [0]


def _fresh_name():
    _counter[0] += 1
    return f"I-waitfix-{_counter[0]}"


def legalize_waits(nc, limit=1):
    for fn in nc.m.functions:
        for blk in fn.blocks:
            out = []
            changed = False
            for inst in blk.instructions:
                si = inst.sync_info
                waits = list(si.on_wait) if si is not None and si.on_wait else []
                if len(waits) > limit:
                    excess = waits[: len(waits) - limit]
                    keep = waits[len(waits) - limit :]
                    for w in excess:
                        out.append(
                            mybir.InstNoOp(
                                name=_fresh_name(),
                                engine=inst.engine,
                                ins=[],
                                outs=[],
                                sync_info=mybir.SyncInfo(on_wait=[w], on_update=[]),
                            )
                        )
                    inst.sync_info = mybir.SyncInfo(
                        on_wait=keep,
                        on_update=list(si.on_update) if si.on_update else [],
                    )
                    changed = True
                out.append(inst)
            if changed:
                blk.instructions = out
    return nc


# ---------------------------------------------------------------------------
# bass program: one core, n_img images
# ---------------------------------------------------------------------------
def _plane(dram_ap):
    """[512,512] DRAM view -> [128, 4, 512] partition-major AP."""
    return dram_ap.rearrange("(k p) w -> p k w", p=128)


def build_nc(n_img=IMG_PER_CORE, mm_dtype=MM_DTYPE):
    f32 = mybir.dt.float32
    nc = bass.Bass()
    x = nc.declare_dram_parameter("x", [n_img, N, N], f32, isOutput=False)
    caspm = nc.declare_dram_parameter("caspm", [2, N, N], f32, isOutput=False)
    emat = nc.declare_dram_parameter("emat", [4, N, N], f32, isOutput=False)
    out = nc.declare_dram_parameter("out", [n_img, N, N], f32, isOutput=True)

    shp = [128, NCHUNK, N]

    with TileContext(nc) as tc:
        with (
            tc.tile_pool(name="wts", bufs=1) as wts,
            tc.tile_pool(name="stg", bufs=2) as stg,
            tc.tile_pool(name="xin", bufs=2) as xin,
            tc.tile_pool(name="mid", bufs=1) as mid,
            tc.tile_pool(name="outp", bufs=2) as outp,
            tc.tile_pool(name="ps", bufs=8, space="PSUM") as ps,
        ):
            # --- one-time: weights (rounded to mm dtype) + E planes ---
            caspt = wts.tile(shp, mm_dtype, tag="caspt")
            casmt = wts.tile(shp, mm_dtype, tag="casmt")
            for i, dst in ((0, caspt), (1, casmt)):
                s = stg.tile(shp, f32, tag="stg")
                nc.sync.dma_start(out=s[:], in_=_plane(caspm[i]))
                nc.vector.tensor_copy(dst[:], s[:])
            et = []
            for i in range(4):
                t = wts.tile(shp, f32, tag=f"e{i}")
                nc.sync.dma_start(out=t[:], in_=_plane(emat[i]))
                et.append(t)

            def acc_mm(bank, terms, mt):
                """bank = sum_terms lhsT_plane[:,k,mt]^T @ rhs[:,k,:]"""
                nterm = len(terms)
                for ti, (plane, rhs) in enumerate(terms):
                    for k in range(NCHUNK):
                        nc.tensor.matmul(
                            bank[:, :],
                            plane[:, k, mt * 128 : (mt + 1) * 128],
                            rhs[:, k, :],
                            start=(ti == 0 and k == 0),
                            stop=(ti == nterm - 1 and k == NCHUNK - 1),
                        )

            for j in range(n_img):
                # --- stage input (cast/round to mm dtype on DVE) ---
                xr = xin.tile(shp, mm_dtype, tag="xr")
                s = stg.tile(shp, f32, tag="stg")
                nc.sync.dma_start(out=s[:], in_=_plane(x[j]))
                nc.vector.tensor_copy(xr[:], s[:])

                # --- S1: S1p = x^T CASP ---
                s1p = mid.tile(shp, mm_dtype, tag="s1p")
                for m in range(NCHUNK):
                    b = ps.tile([128, N], f32, tag="bank")
                    acc_mm(b, [(xr, caspt)], m)
                    nc.scalar.copy(s1p[:, m, :], b[:, :])

                # --- S2 + S3: A = S1p^T CASP; m_i = A * E_i (DVE evac) ---
                mpl = [mid.tile(shp, mm_dtype, tag=f"m{i}", name=f"m{i}") for i in range(4)]
                for m in range(NCHUNK):
                    b = ps.tile([128, N], f32, tag="bank")
                    acc_mm(b, [(s1p, caspt)], m)
                    for i in range(4):
                        nc.vector.tensor_mul(
                            mpl[i][:, m, :], b[:, :], et[i][:, m, :]
                        )

                # --- S4: Ta = m1^T CASP + m2^T CASM; Tb = m3,m4 ---
                ta = mid.tile(shp, mm_dtype, tag="ta")
                tb = mid.tile(shp, mm_dtype, tag="tb")
                for m in range(NCHUNK):
                    ba = ps.tile([128, N], f32, tag="bank")
                    acc_mm(ba, [(mpl[0], caspt), (mpl[1], casmt)], m)
                    nc.scalar.copy(ta[:, m, :], ba[:, :])
                    bb = ps.tile([128, N], f32, tag="bank")
                    acc_mm(bb, [(mpl[2], caspt), (mpl[3], casmt)], m)
                    nc.scalar.copy(tb[:, m, :], bb[:, :])

                # --- S5: out = Ta^T CASP + Tb^T CASM ---
                ot = outp.tile(shp, f32, tag="ot")
                for m in range(NCHUNK):
                    b = ps.tile([128, N], f32, tag="bank")
                    acc_mm(b, [(ta, caspt), (tb, casmt)], m)
                    nc.scalar.copy(ot[:, m, :], b[:, :])
                nc.sync.dma_start(out=_plane(out[j]), in_=ot[:])

    legalize_waits(nc)
    return nc


# ---------------------------------------------------------------------------
# host wrapper
# ---------------------------------------------------------------------------
_nc_cache = {}


def _get_nc(n_img, mm_dtype):
    key = (n_img, str(mm_dtype))
    if key not in _nc_cache:
        _nc_cache[key] = build_nc(n_img, mm_dtype)
    return _nc_cache[key]


def _host_consts():
    n = np.arange(N)
    ang = 2.0 * np.pi * np.outer(n, n) / N
    c = np.cos(ang)
    s = np.sin(ang)
    return np.stack([c + s, c - s]).astype(np.float32)


def _filter_planes(H_real, H_imag):
    """E1..E4 for the row-column DHT convolution theorem (1/N^2 folded in)."""
    Hc = np.asarray(H_real, np.float64) + 1j * np.asarray(H_imag, np.float64)
    idx = (-np.arange(N)) % N
    G = 0.5 * (Hc + np.conj(Hc[np.ix_(idx, idx)]))
    ReH, ImH = np.real(G), np.imag(G)

    def Ru(a):
        return a[idx, :]

    def Rv(a):
        return a[:, idx]

    sc = 1.0 / (N * N)
    E1 = 0.5 * (Ru(ReH) + ReH) * sc
    E2 = 0.5 * (ImH - Ru(ImH)) * sc
    E3 = -0.5 * (Rv(Ru(ImH)) + Rv(ImH)) * sc
    E4 = 0.5 * (Rv(ReH) - Rv(Ru(ReH))) * sc
    return np.stack([E1, E2, E3, E4]).astype(np.float32)


def kernel(x, H_real, H_imag):
    x = np.asarray(x, dtype=np.float32)
    B, C, H, W = x.shape
    assert (H, W) == (N, N) and B * C == N_CORES * IMG_PER_CORE

    emat = _filter_planes(H_real, H_imag)
    caspm = _host_consts()

    xf = np.ascontiguousarray(x.reshape(B * C, N, N))
    nc = _get_nc(IMG_PER_CORE, MM_DTYPE)
    in_maps = [
        {
            "x": xf[i * IMG_PER_CORE : (i + 1) * IMG_PER_CORE],
            "caspm": caspm,
            "emat": emat,
        }
        for i in range(N_CORES)
    ]
    res = run_bass_kernel_spmd(nc, in_maps, list(range(N_CORES)))
    outs = [res.results[i]["out"] for i in range(N_CORES)]
    return np.concatenate(outs, axis=0).reshape(B, C, N, N)
